# revision 19
# baseline (speedup 1.0000x reference)
"""DigitCapsules routing kernel for 8 Trainium2 NeuronCores.

Math: in the reference, u_hat is an explicit broadcast of u_core over the
capsule axis i, so b stays constant along i in every routing iteration,
softmax over i is exactly uniform (1/K), and the whole 3-iteration routing
collapses (exactly, in floating point too) to:

    v[b, i, :] = squash((1/576) * sum_{r,k} x2[b, r, k] * W[b, r, k, :])

broadcast over i = 0..575, where x2 = x.reshape(B, 8, 576).transpose(0, 2, 1).
The i-broadcast is pure replication, done on the host after the gather
(np.broadcast_to) - the device computes and returns only the unique
[4, 16] v rows per core.

Sharding: batch dim B=32 across 8 cores, 4 batches per core (data parallel).

Device structure:
 - wx = [W | x2] packed fp16 on host: per batch 4 full r-tiles of 128 rows
   plus a half tile; two batches share each half tile (64 partitions each)
   so there is no zero padding.  Three DMAs on three independent
   descriptor-generation paths (sync HWDGE / gpsimd SWDGE / scalar HWDGE
   for the small mask) - HWDGE ring generation is serialized, SWDGE is not.
 - G: batch n lands in PSUM partitions 32n..32n+8 via PE column-group
   tiling (q0/q32/q64/q96), two [128, 128] PSUM tiles (pair b0/b1 and
   b2/b3) so the first pair's mask-multiply is not dependency-chained
   behind the second pair's matmuls.  All DMA1-gated matmuls are emitted
   before any DMA2-gated one: PE matmul starts are strictly pc-monotone,
   so a semaphore-stalled matmul blocks every later one.
 - One mask-multiply (fp16, two partition halves) extracts the k-diagonal;
   a one-hot [128, 4] matmul gives T2 [4, 128] fp32; a tiny [4, 16, 8]
   reduce lands T in SBUF.
 - Squash over [4, 16] with two parallel reciprocals (1/(1+norm) folds into
   the coefficient during the Sqrt latency; Rsqrt is banned for accuracy).
   A dummy early Sqrt hoists the 1.3us ACT table load off the critical path.
 - Output: one 256-byte DMA of v [4, 16] fp32.
 - 8 junk matmuls (N=448, operands mostly uninitialized) warm the PE HAM
   clock gate during the DMA wait (PE runs 1.2 GHz until ~3.4us of
   sustained activity).

Perf notes from traces:
 - ~13.0us of the measured window is fixed (measured with a do-nothing
   kernel): bass init, tiny-DMA in/out latencies, and a runtime-injected
   postamble that clears all 254 semaphores one instruction at a time.
 - Input DMA sustains ~20B/ns per SDMA engine; engines 8-15 are slower.
 - Tile tracks PSUM regions by free-dim ranges only -> partition-sliced
   reads of one tile wait on every writer; use separate tiles.
 - tensor_tensor with BOTH operands in PSUM is not allowed; custom DVE ops
   (tensor_tensor_reduce etc.) hard-crash this runtime - avoid.
"""

import numpy as np

import concourse.bacc as bacc
import concourse.mybir as mybir
import concourse.tile as tile
from concourse.bass_utils import run_bass_kernel_spmd

N_CORES = 8
B, C, H, W_ = 32, 8, 24, 24
R = H * W_          # 576 routes
KJ = 128            # fused (j=16, k=8) W column axis, j-major
D = 16
NB = B // N_CORES   # 4 batches per core
WX = KJ + C         # 136 cols per (batch, tile)
FULL_T = 4          # full 128-row r-tiles per batch
# col layout: b0 | b1 | h01 | b2d01 | b2d23 | h23 | b3 | mask
T0 = [0, 136, 272, 408]
T1 = [544, 680, 816, 952]
H01 = 1088
T2 = [1224, 1360, 1496, 1632]
H23 = 1768
T3 = [1904, 2040, 2176, 2312]
MASK_OFF = 2448
COLS = 2576
DMA1_END = 1496                      # b0, b1, h01, b2 d0/d1   (383 KB)
DMA2_END = 2448                      # b2 d2/d3, h23, b3       (244 KB)
RNORM = 1.0 / float(R)
RNORM2 = RNORM * RNORM
RNORM3 = RNORM2 * RNORM
N_JUNK = 8

_cached_nc = None
_last_in_maps = None


def _build():
    nc = bacc.Bacc(trn_type="TRN2")
    f32 = mybir.dt.float32
    f16 = mybir.dt.float16

    wx_h = nc.dram_tensor("wx", [128, COLS], f16, kind="ExternalInput")
    out_h = nc.dram_tensor("out", [NB, D], f32, kind="ExternalOutput")

    with tile.TileContext(nc) as tc:
        with (
            tc.tile_pool(name="sb", bufs=1) as sb,
            tc.tile_pool(name="gps", bufs=1, space="PSUM") as gps,
            tc.tile_pool(name="tps", bufs=1, space="PSUM") as tps,
            tc.tile_pool(name="jps", bufs=1, space="PSUM") as jps,
        ):
            # --- early consts / scratch (all off the critical path) ---
            oneh = sb.tile([128, NB], f16)
            nc.vector.memset(oneh[:], 0.0)
            for n in range(NB):
                nc.vector.memset(oneh[32 * n:32 * n + 8, n:n + 1], 1.0)
            eps_t = sb.tile([NB, 1], f32)
            nc.vector.memset(eps_t[:], 1e-8)
            # junk moving operand deliberately left uninitialized (garbage
            # fp16 is fine, results are discarded); Tile requires at least
            # one writer per tile, so memset only the 8 stationary columns.
            junk_sb = sb.tile([128, 456], f16)
            nc.vector.memset(junk_sb[:, 448:456], 0.5)
            # dummy Sqrt: forces the ACT table loads to execute here (~8us,
            # overlapping the DMA wait) instead of right before the real Sqrt
            dummy = sb.tile([NB, 1], f32)
            nc.scalar.activation(
                dummy[:], eps_t[:], mybir.ActivationFunctionType.Sqrt)

            # G pair tiles: batch n in partitions 32n..32n+8 (n=0,1 in g_lo,
            # n=2,3 in g_hi).  Zero the rows the mask-multiply reads so
            # stale PSUM bits (possibly NaN) never propagate.
            g_lo = gps.tile([128, KJ], f32, tag="g_lo")
            g_hi = gps.tile([128, KJ], f32, tag="g_hi")
            nc.vector.memset(g_lo[0:64, :], 0.0)
            nc.vector.memset(g_hi[64:128, :], 0.0)

            # --- PE warm-up across the DMA wait ---
            junk_ps = jps.tile([8, 448], f32)
            for _ in range(N_JUNK):
                nc.tensor.matmul(
                    junk_ps[:], junk_sb[:, 448:456], junk_sb[:, 0:448],
                    start=True, stop=True,
                )

            # --- input: three DMAs on three descriptor-generation paths ---
            wx_t = sb.tile([128, COLS], f16)
            nc.sync.dma_start(wx_t[:, 0:DMA1_END], wx_h[:, 0:DMA1_END])
            nc.gpsimd.dma_start(
                wx_t[:, DMA1_END:DMA2_END], wx_h[:, DMA1_END:DMA2_END])
            nc.scalar.dma_start(wx_t[:, DMA2_END:COLS], wx_h[:, DMA2_END:COLS])

            mask16 = wx_t[:, MASK_OFF:MASK_OFF + KJ]

            # --- G[32n+k, j*8+k'] = sum_r x2[n, r, k] * W[n, r, j*8+k'] ---
            def mm(n, c0, p0=0, nrow=128, start=False, stop=False):
                g_t = g_lo if n < 2 else g_hi
                nc.tensor.matmul(
                    g_t[32 * n:32 * n + 8, :],
                    wx_t[p0:p0 + nrow, c0 + KJ:c0 + WX],
                    wx_t[p0:p0 + nrow, c0:c0 + KJ],
                    start=start, stop=stop, tile_position=(p0, 32 * n),
                )

            # DMA1-gated matmuls first (pc-monotone starts: a stalled
            # matmul blocks all later ones)
            for d in range(FULL_T):
                mm(0, T0[d], start=(d == 0))
                mm(1, T1[d], start=(d == 0))
            mm(2, T2[0], start=True)
            mm(2, T2[1])
            mm(0, H01, p0=0, nrow=64, stop=True)
            mm(1, H01, p0=64, nrow=64, stop=True)
            # DMA2-gated
            mm(3, T3[0], start=True)
            mm(2, T2[2])
            mm(3, T3[1])
            mm(2, T2[3])
            mm(3, T3[2])
            mm(2, H23, p0=0, nrow=64, stop=True)
            mm(3, T3[3])
            mm(3, H23, p0=64, nrow=64, stop=True)

            # --- k-diagonal mask-mul: the b0/b1 half runs while the PE is
            # still contracting b2/b3 ---
            pm = sb.tile([128, KJ], f16)
            nc.vector.tensor_mul(pm[0:64, :], g_lo[0:64, :], mask16[0:64, :])
            nc.vector.tensor_mul(
                pm[64:128, :], g_hi[64:128, :], mask16[64:128, :])
            # T2[n, j*8+k] = sum_p oneh[p, n] * pm[p, j*8+k]; the k-sum then
            # happens on a tiny [4, 16, 8] reduce that lands T in SBUF (fp32
            # accumulation, no PSUM->SBUF copy)
            t2_ps = tps.tile([NB, KJ], f32)
            nc.tensor.matmul(t2_ps[:], oneh[:], pm[:], start=True, stop=True)
            t_sb = sb.tile([NB, D], f32)
            nc.vector.reduce_sum(
                t_sb[:], t2_ps[:].rearrange("p (j k) -> p j k", j=D),
                axis=mybir.AxisListType.X,
            )

            # --- squash: v = T * (normT/576^3) / ((1+norm) sqrt(norm+eps)),
            #     norm = normT/576^2,  normT = sum_j T^2 ---
            sq = sb.tile([NB, D], f32)
            nc.vector.tensor_mul(sq[:], t_sb[:], t_sb[:])
            normt = sb.tile([NB, 1], f32)
            nc.vector.reduce_sum(normt[:], sq[:], axis=mybir.AxisListType.X)
            q = sb.tile([NB, 1], f32)
            nc.scalar.activation(
                q[:], normt[:], mybir.ActivationFunctionType.Sqrt,
                bias=eps_t[:], scale=RNORM2,
            )
            # 1/((1+norm)q) as two parallel reciprocals: 1/(1+norm) folds
            # into the coefficient during the Sqrt latency; only 1/q remains
            # on the post-Sqrt path.  Emission order = DVE FIFO order: rq
            # before c2 so it issues as soon as the Sqrt lands.
            a1 = sb.tile([NB, 1], f32)
            nc.vector.tensor_scalar(
                out=a1[:], in0=normt[:], scalar1=RNORM2, scalar2=1.0,
                op0=mybir.AluOpType.mult, op1=mybir.AluOpType.add,
            )
            ra = sb.tile([NB, 1], f32)
            nc.vector.reciprocal(ra[:], a1[:])
            rq = sb.tile([NB, 1], f32)
            nc.vector.reciprocal(rq[:], q[:])
            c2 = sb.tile([NB, 1], f32)
            nc.vector.tensor_scalar(
                out=c2[:], in0=normt[:], scalar1=RNORM3, scalar2=ra[:],
                op0=mybir.AluOpType.mult, op1=mybir.AluOpType.mult,
            )
            v_sb = sb.tile([NB, D], f32)
            nc.vector.tensor_scalar(
                out=v_sb[:], in0=t_sb[:], scalar1=c2[:], scalar2=rq[:],
                op0=mybir.AluOpType.mult, op1=mybir.AluOpType.mult,
            )

            nc.sync.dma_start(out_h[:, :], v_sb[:])

    nc.finalize()
    return nc


def _pack_inputs(x, w):
    x = np.ascontiguousarray(np.asarray(x), dtype=np.float32)
    w = np.ascontiguousarray(np.asarray(w), dtype=np.float32)
    x2 = x.reshape(B, C, R).transpose(0, 2, 1)          # [B, R, 8]
    # j-major column packing: wf[b, r, j*8+k] = W[b, r, k, j]
    wf = w.reshape(B, R, C, D).transpose(0, 1, 3, 2).reshape(B, R, KJ)
    blk = np.concatenate([wf, x2], axis=2)              # [B, 576, 136]
    full = (
        blk[:, :512].reshape(B, FULL_T, 128, WX).transpose(0, 1, 2, 3)
    )                                                   # [B, 4, 128, 136]
    half = blk[:, 512:]                                 # [B, 64, 136]
    p = np.arange(128)[:, None]
    c = np.arange(KJ)[None, :]
    mask = (((p % 32) < 8) & ((c % 8) == (p % 32))).astype(np.float32)
    tiles = [T0, T1, T2, T3]
    in_maps = []
    for core in range(N_CORES):
        bs = [core * NB + n for n in range(NB)]
        wx = np.zeros((128, COLS), np.float32)
        for n in range(NB):
            for d in range(FULL_T):
                wx[:, tiles[n][d]:tiles[n][d] + WX] = full[bs[n], d]
        wx[0:64, H01:H01 + WX] = half[bs[0]]
        wx[64:128, H01:H01 + WX] = half[bs[1]]
        wx[0:64, H23:H23 + WX] = half[bs[2]]
        wx[64:128, H23:H23 + WX] = half[bs[3]]
        wx[:, MASK_OFF:MASK_OFF + KJ] = mask
        in_maps.append({"wx": np.ascontiguousarray(wx.astype(np.float16))})
    return in_maps


def kernel(x, route_weights):
    global _cached_nc, _last_in_maps
    if _cached_nc is None:
        _cached_nc = _build()
    nc = _cached_nc

    in_maps = _pack_inputs(x, route_weights)
    _last_in_maps = in_maps

    res = run_bass_kernel_spmd(nc, in_maps, core_ids=list(range(N_CORES)))
    v = np.concatenate([r["out"] for r in res.results], axis=0)   # [32, 16]
    return np.ascontiguousarray(
        np.broadcast_to(v[:, None, :], (B, R, D)).astype(np.float32)
    )
